# revision 3
# baseline (speedup 1.0000x reference)
"""NT-Xent loss on 8 Trainium2 NeuronCores — symmetric fp8 version.

Full inputs in, full (scalar) output out. The 8192x8192 similarity
matrix is symmetric: each 1024x1024 block is computed once. Core c
computes blocks (c, c+L mod 8) for L=0..4 (block L=4 is the positive
pair block, computed redundantly by both ends so no exchange is
needed for it). Inputs are row-rotated per core so the single SPMD
program sees its own rows at local positions 0..1023.

Row sums of exp(sim/T) for the core's own rows come from the ACT
accumulator; column sums of blocks L=1..3 (contributions to OTHER
cores' rows) come from ones-matmuls into a persistent psum region,
are rotated into global block order by a per-core permutation matmul,
and exchanged with a ReduceScatter (DRAM bounce). All block L=4 exps
run AFTER the ReduceScatter is dispatched so the collective hides
under ~10us of ACT work. Matmuls run in fp8e4 DoubleRow mode (K=256
in one pass at 2x rate); features are quantized to fp8 on the host
(raw values ~N(0,1) suit e4m3) and normalized on-device from the
quantized values.
"""
import numpy as np
import ml_dtypes

import concourse.bass as bass  # noqa: F401
import concourse.tile as tile
import concourse.bacc as bacc_mod
from concourse import bacc, mybir
from concourse.bass_utils import run_bass_kernel_spmd
from concourse.hw_specs import get_activation_tables as _real_tables

B, D = 4096, 256
N = 2 * B                # 8192 rows/cols of sim matrix
NCORES = 8
RPC = N // NCORES        # 1024 rows per core
TEMP = 0.07
SCALE = 1.0 / TEMP
KG = 2                   # contraction groups: D = 256 = 2 * 128
NBLK = 5                 # column blocks L=0..4 per core
NCOL = NBLK * RPC        # 5120 columns per core
MT = RPC // 128          # 8 M-tiles per core
NEG = -1.0e9
PW = 1536                # main-loop psum group width

AF = mybir.ActivationFunctionType
ALU = mybir.AluOpType
AX = mybir.AxisListType
PM = mybir.MatmulPerfMode
f32 = mybir.dt.float32
bf16 = mybir.dt.bfloat16
fp8 = mybir.dt.float8e4

_CACHE = {}


def _pinned_tables(arch):
    """Keep Exp/Ln only in natural_log_exp_and_others so the act-table
    insertion pass picks one set for the whole kernel (no reload)."""
    tables = _real_tables(arch)
    out = {}
    for name, funcs in tables.items():
        if name != "natural_log_exp_and_others":
            funcs = {f for f in funcs if f.name not in ("Exp", "Ln")}
        out[name] = funcs
    return out


def _build_nc():
    bacc_mod.get_activation_tables = _pinned_tables
    nc = bacc.Bacc("TRN2", target_bir_lowering=False, debug=False,
                   enable_asserts=False, num_devices=NCORES,
                   num_swdge_queues=2)

    zt_d = nc.dram_tensor("zt", [KG, 128, NCOL], fp8, kind="ExternalInput")
    cones8_d = nc.dram_tensor("cones8", [128, KG, 128], fp8,
                              kind="ExternalInput")
    conesb_d = nc.dram_tensor("conesb", [128, 128], bf16,
                              kind="ExternalInput")
    vonesb_d = nc.dram_tensor("vonesb", [128, 1], bf16,
                              kind="ExternalInput")
    vonesf_d = nc.dram_tensor("vonesf", [128, 1], f32, kind="ExternalInput")
    negid_d = nc.dram_tensor("negid", [128, 128], f32, kind="ExternalInput")
    perm_d = nc.dram_tensor("perm", [96, 8], bf16, kind="ExternalInput")
    out_d = nc.dram_tensor("out", [1, 1], f32, kind="ExternalOutput")

    with tile.TileContext(nc) as tc:
        with (
            tc.tile_pool(name="singles", bufs=1) as singles,
            tc.tile_pool(name="sqp", bufs=2) as sqp,
            tc.tile_pool(name="invp", bufs=2) as invp,
            tc.tile_pool(name="expp", bufs=2) as expp,
            tc.tile_pool(name="ps", bufs=2, space="PSUM") as ps,
            tc.tile_pool(name="csp", bufs=1, space="PSUM") as csp,
            tc.tile_pool(name="dram", bufs=1, space="DRAM") as dram,
        ):
            # --- constants (SWDGE ring) ---
            cones8 = singles.tile([128, KG, 128], fp8, tag="cones8")
            nc.gpsimd.dma_start(out=cones8, in_=cones8_d.ap())
            conesb = singles.tile([128, 128], bf16, tag="conesb")
            nc.gpsimd.dma_start(out=conesb, in_=conesb_d.ap())
            vonesb = singles.tile([128, 1], bf16, tag="vonesb")
            nc.gpsimd.dma_start(out=vonesb, in_=vonesb_d.ap())
            vonesf = singles.tile([128, 1], f32, tag="vonesf")
            nc.gpsimd.dma_start(out=vonesf, in_=vonesf_d.ap())
            negid = singles.tile([128, 128], f32, tag="negid")
            nc.gpsimd.dma_start(out=negid, in_=negid_d.ap())
            perm = singles.tile([96, 8], bf16, tag="perm")
            nc.gpsimd.dma_start(out=perm, in_=perm_d.ap())

            # --- features: 3 chunk DMAs over 3 rings ---
            zt = singles.tile([128, KG, NCOL], fp8, tag="zt")
            zt_ap = zt_d.ap()
            for (w, off), eng in zip(((2048, 0), (2048, 2048), (1024, 4096)),
                                     (nc.sync, nc.scalar, nc.gpsimd)):
                eng.dma_start(
                    out=zt[:, :, off:off + w],
                    in_=zt_ap[:, :, off:off + w].rearrange("k p c -> p k c"))

            nf = singles.tile([128, KG, NCOL], fp8, tag="nf")
            sums = singles.tile([128, MT * 4], f32, tag="sums")

            # persistent colsum accumulators: L at partition 32*(L-1)
            cs_ps = csp.tile([128, 1024], f32, tag="cs", name="cs_ps")
            nc.vector.memset(cs_ps, 0.0)
            vsb = singles.tile([96, 1024], bf16, tag="vsb")
            nc.gpsimd.memset(vsb, 0.0)

            def normalize(w, off):
                sq = sqp.tile([128, KG, w], fp8, tag=f"sq{w}",
                              name=f"sq{off}")
                nc.vector.tensor_mul(sq, zt[:, :, off:off + w],
                                     zt[:, :, off:off + w])
                nn = ps.tile([128, PW], f32, tag="A", name=f"nn{off}")
                for n in range(w // 512):
                    nc.tensor.matmul(
                        nn[:, 512 * n:512 * (n + 1)], cones8,
                        sq[:, :, 512 * n:512 * (n + 1)],
                        start=True, stop=True, perf_mode=PM.DoubleRow,
                        skip_group_check=True)
                lnv = invp.tile([128, w], f32, tag=f"lnv{w}",
                                name=f"lnv{off}")
                nc.scalar.activation(lnv, nn[:, 0:w], AF.Ln)
                inv = invp.tile([128, w], f32, tag=f"inv{w}",
                                name=f"inv{off}")
                nc.scalar.activation(inv, lnv, AF.Exp, scale=-0.5)
                for kg in range(KG):
                    nc.vector.tensor_mul(nf[:, kg, off:off + w],
                                         zt[:, kg, off:off + w], inv)

            for w, off in ((1536, 0), (1536, 1536), (1536, 3072),
                           (512, 4608)):
                normalize(w, off)

            fin = singles.tile([128, 2], f32, tag="fin")

            # --- positive term (cols 0:1024 vs 4096:5120) ---
            tmp_pos = sqp.tile([128, KG, RPC], bf16, tag="tpos")
            for kg in range(KG):
                nc.vector.tensor_mul(tmp_pos[:, kg, :],
                                     nf[:, kg, 0:RPC],
                                     nf[:, kg, 4 * RPC:5 * RPC])
            pos_ps = ps.tile([128, PW], f32, tag="A", name="pos_ps")
            for n in range(RPC // 512):
                for kg in range(KG):
                    nc.tensor.matmul(
                        pos_ps[:, 512 * n:512 * (n + 1)], conesb,
                        tmp_pos[:, kg, 512 * n:512 * (n + 1)],
                        start=(kg == 0), stop=(kg == KG - 1),
                        skip_group_check=True)
            nc.vector.tensor_reduce(fin[:, 1:2], pos_ps[:, 0:RPC],
                                    axis=AX.X, op=ALU.add)

            # --- main loop: per m, groups T0/T1 (1536) + T2 (1024) over
            # cols 0..4095; colsums of blocks L=1,2,3 accumulate in cs_ps
            def lhsT(m):
                return nf[:, :, 128 * m:128 * m + 128]

            def sim_mm(dst, m, coff, w):
                for n in range(w // 512):
                    nc.tensor.matmul(
                        dst[:, 512 * n:512 * (n + 1)], lhsT(m),
                        nf[:, :, coff + 512 * n:coff + 512 * (n + 1)],
                        start=True, stop=True, perf_mode=PM.DoubleRow,
                        skip_group_check=True)

            def colsum(etile, eoff, L, csoff, m):
                """cs_ps[32*(L-1), csoff:csoff+512] += colsum of
                etile[:, eoff:eoff+512] (rows of m-tile m)."""
                p0 = 32 * (L - 1)
                nc.tensor.matmul(
                    cs_ps[p0:p0 + 1, csoff:csoff + 512],
                    vonesb, etile[:, eoff:eoff + 512],
                    start=(m == 0), stop=(m == MT - 1),
                    tile_position=(0, p0),
                    skip_group_check=True)

            def exp_tile(src, w, idx, name):
                e = expp.tile([128, w], bf16, tag=f"e{w}", name=name)
                nc.scalar.activation(e, src[:, 0:w], AF.Exp, scale=SCALE,
                                     accum_out=sums[:, idx:idx + 1])
                return e

            for m in range(MT):
                T0 = ps.tile([128, PW], f32, tag="A", name=f"T0_{m}")
                sim_mm(T0, m, 0, 1536)
                sl = T0[:, 128 * m:128 * m + 128]
                nc.vector.tensor_add(sl, sl, negid)
                T1 = ps.tile([128, PW], f32, tag="A", name=f"T1_{m}")
                sim_mm(T1, m, 1536, 1536)
                eT0 = exp_tile(T0, 1536, 4 * m + 0, f"eT0_{m}")
                T2 = ps.tile([128, PW], f32, tag="A", name=f"T2_{m}")
                sim_mm(T2, m, 3072, 1024)
                colsum(eT0, 1024, 1, 0, m)        # L1 cols 0:512
                eT1 = exp_tile(T1, 1536, 4 * m + 1, f"eT1_{m}")
                colsum(eT1, 0, 1, 512, m)         # L1 cols 512:1024
                colsum(eT1, 512, 2, 0, m)         # L2 cols 0:512
                colsum(eT1, 1024, 2, 512, m)      # L2 cols 512:1024
                eT2 = exp_tile(T2, 1024, 4 * m + 2, f"eT2_{m}")
                colsum(eT2, 0, 3, 0, m)           # L3 cols 0:512
                colsum(eT2, 512, 3, 512, m)       # L3 cols 512:1024

            # --- permute colsums into global block order, ReduceScatter
            nc.vector.tensor_copy(vsb, cs_ps[0:96, :])
            p_ps = ps.tile([128, PW], f32, tag="A", name="p_ps")
            for h in range(2):
                nc.tensor.matmul(
                    p_ps[0:8, 512 * h:512 * (h + 1)], perm,
                    vsb[:, 512 * h:512 * (h + 1)],
                    start=True, stop=True, skip_group_check=True)
            pout = singles.tile([8, 1024], f32, tag="pout")
            nc.vector.tensor_copy(pout, p_ps[0:8, 0:1024])
            cc_in = dram.tile([8, 1024], f32)
            cc_out = dram.tile([1, 1024], f32)
            nc.sync.dma_start(out=cc_in, in_=pout)
            nc.gpsimd.collective_compute(
                "ReduceScatter", ALU.add,
                replica_groups=[list(range(NCORES))],
                ins=[cc_in[:].opt()], outs=[cc_out[:].opt()])
            rsb = singles.tile([128, MT], f32, tag="rsb")
            nc.sync.dma_start(
                out=rsb,
                in_=cc_out[:].rearrange("one (m p) -> (one p) m", p=128))

            # --- tail: pair-block (L=4) exps hide the collective ---
            for m in range(MT):
                T = ps.tile([128, PW], f32, tag="A", name=f"P_{m}")
                sim_mm(T, m, 4096, 1024)
                exp_tile(T, 1024, 4 * m + 3, f"eP_{m}")

            # --- finish: lse per row, reduce ---
            own = singles.tile([128, MT], f32, tag="own")
            nc.vector.tensor_reduce(
                own, sums.rearrange("p (m g) -> p m g", g=4),
                axis=AX.X, op=ALU.add)
            tot = singles.tile([128, MT], f32, tag="tot")
            nc.vector.tensor_add(tot, own, rsb)
            lse8 = singles.tile([128, MT], f32, tag="lse8")
            nc.scalar.activation(lse8, tot, AF.Ln)
            nc.vector.tensor_reduce(fin[:, 0:1], lse8, axis=AX.X,
                                    op=ALU.add)

            fin_ps = ps.tile([128, PW], f32, tag="A", name="fin_ps")
            nc.tensor.matmul(fin_ps[0:1, 0:2], vonesf, fin,
                             start=True, stop=True, skip_group_check=True)
            # fin_ps[0,0] = sum_p lse_p ; fin_ps[0,1] = 128 * sum_i pos_i
            possc = singles.tile([1, 1], f32, tag="possc")
            nc.vector.tensor_scalar_mul(possc, fin_ps[0:1, 1:2],
                                        SCALE / 128.0)
            outv = singles.tile([1, 1], f32, tag="outv")
            nc.vector.tensor_sub(outv, fin_ps[0:1, 0:1], possc)
            nc.sync.dma_start(out=out_d.ap(), in_=outv)

    nc.compile()
    return nc


def _get_nc():
    if "nc" not in _CACHE:
        _CACHE["nc"] = _build_nc()
    return _CACHE["nc"]


def _in_maps(z_i, z_j):
    feats = np.concatenate([np.asarray(z_i, dtype=np.float32),
                            np.asarray(z_j, dtype=np.float32)], axis=0)
    cones8 = np.ones((128, KG, 128), dtype=ml_dtypes.float8_e4m3)
    conesb = np.ones((128, 128), dtype=ml_dtypes.bfloat16)
    vonesb = np.ones((128, 1), dtype=ml_dtypes.bfloat16)
    vonesf = np.ones((128, 1), dtype=np.float32)
    negid = (NEG * np.eye(128)).astype(np.float32)
    maps = []
    for c in range(NCORES):
        zr = np.roll(feats, -RPC * c, axis=0)[:NCOL]     # [5120, 256]
        zq = zr.T.astype(ml_dtypes.float8_e4m3)          # [256, 5120]
        zt = np.ascontiguousarray(zq.reshape(KG, 128, NCOL))
        perm = np.zeros((96, 8), dtype=ml_dtypes.bfloat16)
        for L in (1, 2, 3):
            perm[32 * (L - 1), (c + L) % NCORES] = 1.0
        maps.append({"zt": zt, "cones8": cones8, "conesb": conesb,
                     "vonesb": vonesb, "vonesf": vonesf, "negid": negid,
                     "perm": perm})
    return maps


def kernel(z_i, z_j, _trace=False, _trace_kwargs=None):
    nc = _get_nc()
    maps = _in_maps(z_i, z_j)
    res = run_bass_kernel_spmd(nc, maps, core_ids=list(range(NCORES)),
                               trace=_trace, **(_trace_kwargs or {}))
    total = sum(float(res.results[c]["out"][0, 0]) for c in range(NCORES))
    out = np.array(np.float32(total / N))
    if _trace:
        kernel._last_result = res
    return out
